# revision 37
# baseline (speedup 1.0000x reference)
"""Trainium2 Bass kernel for Encoder+RegLSTM (embedding lookup -> LSTM -> mask).

Strategy (data-parallel over batch, 8 cores x 8 sequences):
  - The reference's sort-by-length + unsort is an exact identity (the LSTM is
    elementwise in batch), so it is skipped.
  - Embedding gather via SWDGE dma_gather(transpose=True) from a bf16 copy of
    the table, landing directly in x^T layout [E(part), token].
  - Input projection (x @ W_ih^T + b) computed chunk-wise as bf16 matmuls
    accumulating into PSUM (gates^T layout: gate-dim on partitions).
  - LSTM recurrence: per step, 16 weight-stationary bf16 matmuls accumulate
    W_hh @ h_{t-1} on top of the preloaded input projection in PSUM.
  - Cell update is restructured to minimize the serial dependency ring:
      gate m-layout [f, g, i, o]; f and i rows pre-halved (host) so one Tanh
      gives T_m = tanh(pre_m/2), i.e. sigma_m = (1+T_m)/2.  State kept as
      C = 2c in SBUF (keeps the DVE chain free of PSUM-group semaphores):
      sigma_f = 0.5*Tf+0.5 (tensor_scalar), t2 = sigma_f*C_prev,
      t1 = (1+Ti)*Tg (fused scalar_tensor_tensor), C = t1+t2,
      tanh(c) = Tanh(C, scale=0.5), h = sigma_o*tanh(c) written once as
      bf16 (history slice doubles as the next step's matmul operand).
    Tanh(f) is issued first so the t2 path runs during the matmul stream;
    Tanh(g,i) follows back-to-back; sigmoid(o) is off the critical path.
    Bias is accumulated into PSUM by K=1 matmuls during the pre-GEMM.
  - h written once per step as bf16 into the history tile; the same slice is
    the next step's matmul moving operand and the chunk's DMA-out source.
"""

import os
import sys

os.environ.setdefault("TILE_EXHAUSTIVE_MEMORY_SHARE_CHECK", "1")
sys.path.insert(0, "/opt/trn_rl_repo")

import numpy as np
import ml_dtypes

import concourse.tile as tile
from concourse import bacc, mybir, library_config
from concourse import bass_utils

BF16 = mybir.dt.bfloat16
F32 = mybir.dt.float32
I16 = mybir.dt.int16

V, E, H = 32000, 256, 256
G4 = 4 * H  # 1024 gate dims, 8 m-tiles of 128
NCORES = 8
BL = 8  # batch per core
S_FULL = 2048
PCHUNK = 32  # steps per PSUM chunk (4 banks)
GS = 64  # steps per gather chunk (dma_gather breaks somewhere in 512<n_idx<=1024)

ALU = mybir.AluOpType
ACTF = mybir.ActivationFunctionType


def build_nc(S=S_FULL, gs=GS):
    NP = S // PCHUNK
    NG = S // gs
    NIDX = gs * BL  # indices per gather chunk
    SPG = gs // PCHUNK  # pchunks per gather chunk

    nc = bacc.Bacc("TRN2", target_bir_lowering=False, debug=False)

    idx_d = nc.dram_tensor("idx", [NG, 128, NIDX // 16], I16, kind="ExternalInput")
    emb_d = nc.dram_tensor("embed", [V, E], BF16, kind="ExternalInput")
    wih_d = nc.dram_tensor("wih", [2, 128, G4], BF16, kind="ExternalInput")
    whh_d = nc.dram_tensor("whh", [2, 128, G4], BF16, kind="ExternalInput")
    bias_d = nc.dram_tensor("bias", [1, G4], BF16, kind="ExternalInput")
    out_d = nc.dram_tensor("out", [S // PCHUNK, 128, PCHUNK * 16], BF16, kind="ExternalOutput")

    with tile.TileContext(nc) as tc:
        wpool = tc.alloc_tile_pool(name="w", bufs=1)
        ipool = tc.alloc_tile_pool(name="ip", bufs=2)
        xpool = tc.alloc_tile_pool(name="xp", bufs=2)
        pspool = tc.alloc_tile_pool(name="ps", bufs=2, space="PSUM")
        cpool = tc.alloc_tile_pool(name="cp", bufs=3)
        tpool = tc.alloc_tile_pool(name="tp", bufs=3)
        histpool = tc.alloc_tile_pool(name="hist", bufs=3)

        nc.gpsimd.load_library(library_config.mlp)

        # --- weights to SBUF ---
        wih = wpool.tile([128, 2, G4], BF16)
        nc.sync.dma_start(out=wih[:, :, :], in_=wih_d.ap().rearrange("c p n -> p c n"))
        whh = wpool.tile([128, 2, G4], BF16)
        nc.sync.dma_start(out=whh[:, :, :], in_=whh_d.ap().rearrange("c p n -> p c n"))
        bias = wpool.tile([1, G4], BF16)
        nc.sync.dma_start(out=bias[:, :], in_=bias_d[:, :])
        ones = wpool.tile([1, 256], BF16)
        nc.vector.memset(ones[:, :], 1.0)

        # --- initial state ---
        h0 = wpool.tile([128, 16], BF16)
        nc.vector.memset(h0[:, :], 0.0)
        c0 = wpool.tile([128, 2, BL], F32)
        nc.vector.memset(c0[:, :, :], 0.0)

        hist_prev = None  # previous chunk's history tile
        cS = c0[:, :, :]  # C = 2c state; lives in the psum f-slot after step 0

        xts = [None] * NG
        pss = [None] * NP

        def emit_gather(g):
            idx_sb = ipool.tile([128, NIDX // 16], I16)
            nc.sync.dma_start(out=idx_sb[:, :], in_=idx_d[g, :, :])
            xts[g] = xpool.tile([128, 2, NIDX], BF16, tag="xt", name="xt")
            nc.gpsimd.dma_gather(
                xts[g][:, :, :],
                emb_d[:, :],
                idx_sb[:, :],
                NIDX,
                NIDX,
                E,
                transpose=True,
            )

        # preGEMM piece list: per bank, even-m c0 (start=True) first
        PIECES = []
        for bk in range(4):
            me, mo = 2 * bk, 2 * bk + 1
            PIECES += [(me, 0, True), (mo, 0, False), (me, 1, False), (mo, 1, False)]
        PIECES += [(m, "bias", False) for m in range(8)]

        def emit_pregemm_piece(pc, i):
            if i == 0:
                pss[pc] = pspool.tile([128, PCHUNK * 64], F32, tag="psc", name="psc")
            ps = pss[pc]
            g, t0 = (pc * PCHUNK) // gs, (pc * PCHUNK) % gs
            m, c, st = PIECES[i]
            if c == "bias":
                return nc.tensor.matmul(
                    ps[:, m * 256 : (m + 1) * 256],
                    bias[0:1, m * 128 : (m + 1) * 128],
                    ones[0:1, 0:256],
                    start=False,
                    stop=False,
                    skip_group_check=True,
                )
            else:
                return nc.tensor.matmul(
                    ps[:, m * 256 : (m + 1) * 256],
                    wih[:, c, m * 128 : (m + 1) * 128],
                    xts[g][:, c, t0 * BL : (t0 + PCHUNK) * BL],
                    start=st,
                    stop=False,
                    skip_group_check=True,
                )

        def emit_step(t, hist):
            nonlocal cS, hist_prev
            pc, tl = t // PCHUNK, t % PCHUNK
            ps = pss[pc]
            # previous h: slice of this chunk's hist (tl>0), previous chunk's
            # hist (tl==0), or the zero tile at t==0
            if t == 0:
                hp = h0[:, :].rearrange("p (c b) -> p c b", c=2)
            elif tl == 0:
                hp = hist_prev[:, (PCHUNK - 1) * 16 : PCHUNK * 16].rearrange(
                    "p (c b) -> p c b", c=2
                )
            else:
                hp = hist[:, (tl - 1) * 16 : tl * 16].rearrange("p (c b) -> p c b", c=2)

            # recurrent matmuls: m-major [f f g g i i o o], c-interleaved
            for m in range(8):
                for c2 in range(2):
                    nc.tensor.matmul(
                        ps[:, m * 256 + tl * BL : m * 256 + (tl + 1) * BL],
                        whh[:, c2, m * 128 : (m + 1) * 128],
                        hp[:, c2, :],
                        start=False,
                        stop=(m == 7 and c2 == 1),
                        skip_group_check=True,
                    )
            psv = ps[:, :].rearrange("p (m t b) -> p m t b", m=8, t=PCHUNK)

            # T = tanh of f (early, feeds t2) then g,i
            T = tpool.tile([128, 6 * BL], F32, tag="T")
            Tv = T[:, :].rearrange("p (m b) -> p m b", m=6)
            nc.scalar.activation(Tv[:, 0:2, :], psv[:, 0:2, tl, :], ACTF.Tanh)
            nc.scalar.activation(Tv[:, 2:6, :], psv[:, 2:6, tl, :], ACTF.Tanh)
            # sigmoid(o) off the critical path
            so = tpool.tile([128, 2 * BL], F32, tag="so")
            sov = so[:, :].rearrange("p (m b) -> p m b", m=2)
            nc.scalar.activation(sov[:, :, :], psv[:, 6:8, tl, :], ACTF.Sigmoid)

            # cell update (C = 2c): t2 = sigma_f * C_prev off the spine via
            # tensor_scalar + mul; t1 fused STT; C = t1 + t2 in SBUF
            u = tpool.tile([128, 2 * BL], BF16, tag="u")
            uv = u[:, :].rearrange("p (c b) -> p c b", c=2)
            nc.vector.tensor_scalar(
                uv[:, :, :], Tv[:, 0:2, :], 0.5, 0.5, ALU.mult, ALU.add
            )
            t2 = tpool.tile([128, 2 * BL], BF16, tag="t2")
            t2v = t2[:, :].rearrange("p (c b) -> p c b", c=2)
            t2_inst = nc.vector.tensor_mul(t2v[:, :, :], uv[:, :, :], cS)
            t1 = tpool.tile([128, 2 * BL], BF16, tag="t1")
            t1v = t1[:, :].rearrange("p (c b) -> p c b", c=2)
            t1_inst = nc.vector.scalar_tensor_tensor(
                t1v[:, :, :], Tv[:, 4:6, :], 1.0, Tv[:, 2:4, :], ALU.add, ALU.mult
            )
            cN = cpool.tile([128, 2 * BL], F32, tag="c")
            cNv = cN[:, :].rearrange("p (c b) -> p c b", c=2)
            cn_inst = nc.vector.tensor_add(cNv, t1v[:, :, :], t2v[:, :, :])
            # tanh(c) = tanh(0.5 * C); C in SBUF keeps the whole DVE chain
            # free of PSUM-group semaphores (back-to-back ops)
            tc = cpool.tile([128, 2 * BL], F32, tag="tc")
            tcv = tc[:, :].rearrange("p (c b) -> p c b", c=2)
            nc.scalar.activation(tcv[:, :, :], cNv, ACTF.Tanh, scale=0.5)
            # h = sigma_o * tanh(c), written once as bf16
            hslice = hist[:, tl * 16 : (tl + 1) * 16].rearrange("p (c b) -> p c b", c=2)
            nc.vector.tensor_mul(hslice, sov[:, :, :], tcv[:, :, :])
            cS = cNv
            return t2_inst

        def emit_out(pc, hist):
            nc.sync.dma_start(out=out_d[pc, :, :], in_=hist[:, :])

        emit_gather(0)
        for i in range(len(PIECES)):
            emit_pregemm_piece(0, i)
        for pc in range(NP):
            if pc % SPG == 0 and pc // SPG + 1 < NG:
                emit_gather(pc // SPG + 1)
            hist = histpool.tile([128, PCHUNK * 16], BF16)
            for s in range(PCHUNK):
                t = pc * PCHUNK + s
                cn_inst = emit_step(t, hist)
                if pc + 1 < NP and s < len(PIECES):
                    pinst = emit_pregemm_piece(pc + 1, s)
                    tile.add_dep_helper(pinst.ins, cn_inst.ins, sync=True, reason="pe warmup")
            emit_out(pc, hist)
            hist_prev = hist

        for p in (histpool, tpool, cpool, pspool, xpool, ipool, wpool):
            p.release()

    nc.compile()
    return nc


def make_inputs(text_inputs, embed, W_ih, W_hh, b_ih, b_hh, S=S_FULL, gs=GS):
    """Host-side marshaling into per-core in_maps."""
    NG = S // gs
    NIDX = gs * BL
    tok = np.asarray(text_inputs).astype(np.int32)
    emb_bf = np.asarray(embed).astype(ml_dtypes.bfloat16)
    # permute gate rows [i, f, g, o] -> [f, g, i, o] (kernel's psum layout);
    # double g rows (tanh(g) = 2*sigma(2g)-1, computed via sigmoid)
    perm = np.concatenate(
        [np.arange(256, 512), np.arange(512, 768), np.arange(0, 256), np.arange(768, 1024)]
    )
    scale = np.concatenate(
        [np.full(256, 0.5), np.full(256, 1.0), np.full(256, 0.5), np.full(256, 1.0)]
    ).astype(np.float32)
    W_ih = np.asarray(W_ih)[perm] * scale[:, None]
    W_hh = np.asarray(W_hh)[perm] * scale[:, None]
    bsum = (np.asarray(b_ih) + np.asarray(b_hh))[perm] * scale
    wih_t = np.ascontiguousarray(W_ih.T).reshape(2, 128, G4).astype(ml_dtypes.bfloat16)
    whh_t = np.ascontiguousarray(W_hh.T).reshape(2, 128, G4).astype(ml_dtypes.bfloat16)
    bias = bsum.reshape(1, G4).astype(ml_dtypes.bfloat16)

    in_maps = []
    for m in range(NCORES):
        tc_ = tok[m * BL : (m + 1) * BL, :S]
        idx = np.empty((NG, 128, NIDX // 16), np.int16)
        for g in range(NG):
            flat = tc_[:, g * gs : (g + 1) * gs].T.reshape(-1)  # (t, b) order
            wrapped = flat.reshape(-1, 16).T.astype(np.int16)  # [16, NIDX//16]
            idx[g] = np.tile(wrapped, (8, 1))
        in_maps.append(
            {"idx": idx, "embed": emb_bf, "wih": wih_t, "whh": whh_t, "bias": bias}
        )
    return in_maps


def unpermute_out(raw):
    """[NP, 128, PCHUNK*16] (ch, p, (tl, c, b)) -> [BL, S, 256]"""
    NP = raw.shape[0]
    v = raw.astype(np.float32).reshape(NP, 128, PCHUNK, 2, BL)  # ch, p, tl, c, b
    v = v.transpose(4, 0, 2, 3, 1)  # b, ch, tl, c, p
    return np.ascontiguousarray(v).reshape(BL, NP * PCHUNK, 2 * 128)


_nc_cache = {}


def _get_nc(S=S_FULL, gs=GS):
    key = (S, gs)
    if key not in _nc_cache:
        _nc_cache[key] = build_nc(S, gs)
    return _nc_cache[key]


def kernel(text_inputs, mask_input, len_seq, embed, W_ih, W_hh, b_ih, b_hh):
    nc = _get_nc()
    in_maps = make_inputs(text_inputs, embed, W_ih, W_hh, b_ih, b_hh)
    try:
        res = bass_utils.run_bass_kernel_spmd(nc, in_maps, core_ids=list(range(NCORES)))
    except Exception:
        # transient device-state failures recover on retry
        res = bass_utils.run_bass_kernel_spmd(nc, in_maps, core_ids=list(range(NCORES)))
    out = np.concatenate(
        [unpermute_out(res.results[m]["out"]) for m in range(NCORES)], axis=0
    )
    mask = np.asarray(mask_input)
    if not np.all(mask == 1.0):
        out = out * mask[..., None]
    return out.astype(np.float32)
